# revision 4
# baseline (speedup 1.0000x reference)
"""MultiHeadAttention Trainium2 kernel (8 NeuronCores).

Reference computation (B=2, S=2048, D=1024, H=16, depth=64):
    qh = split_heads(q @ Wq.T + bq); kh, vh likewise
    logits = qh @ kh.T / sqrt(64) + mask*1e-9
    weights = softmax(logits); attn = weights @ vh
    out = merge_heads(attn) @ Wo.T + bo
    returns (out, weights)

Sharding: 8 cores = 2 batches x 4 head-groups (4 heads/core).
Each core computes its batch's projections restricted to its 4 heads
(column-parallel Wq/Wk/Wv, row-parallel Wo) and a partial `out` for its
batch; host sums the 4 partials per batch and adds bo.

Device pipeline per (head, q-block of 128 rows):
  PE:   logits[128, 2048] = qhT.T @ khT   (fp16, K=64, folded 1/8 scale)
  ACT:  wexp = exp(logits - 8) -> fp16, with fp32 row-sum accumulation
  DVE:  recip = 1/sums ; wnorm = wexp * recip -> fp32 -> DMA (weights out)
  ACT-issued DMA xbar transposes: wexp[128q,128k] tiles -> wbt[k, q]
  PE:   attnT[64, q] += vh_tile.T @ wbt   (fp16, unnormalized)
  late: attn scaled by recip (broadcast via small DRAM bounce), fp16
  PE:   out_partial[t, :] = attnT.T @ WoT_slice (fp16)

Note: `mask` enters the reference as `mask * 1e-9` with mask ~ U[0,1);
its effect on the softmax is O(1e-9) relative — far below fp32 noise —
so the kernel omits it. bq/bk/bv are applied on-device; bo on host.
"""

import numpy as np

P = 128
S = 2048
D = 1024
HEADS_PER_CORE = 4
DEPTH = 64
KB = S // P       # 16 k-blocks
QB = S // P       # 16 q-blocks
KT = D // P       # 8 contraction tiles for projections
OCOLS = HEADS_PER_CORE * DEPTH  # 256 per-core projection columns
EXP_BIAS = -8.0

_CACHED = {}


def _build():
    import concourse.bacc as bacc
    import concourse.bass as bass
    import concourse.tile as tile
    from concourse import mybir

    f16 = mybir.dt.float16
    f32 = mybir.dt.float32

    nc = bacc.Bacc("TRN2", target_bir_lowering=False)

    qT = nc.dram_tensor("qT", [D, S], f16, kind="ExternalInput")
    kT = nc.dram_tensor("kT", [D, S], f16, kind="ExternalInput")
    vT = nc.dram_tensor("vT", [D, S], f16, kind="ExternalInput")
    wq = nc.dram_tensor("wq", [D, OCOLS], f16, kind="ExternalInput")
    wk = nc.dram_tensor("wk", [D, OCOLS], f16, kind="ExternalInput")
    wv = nc.dram_tensor("wv", [D, OCOLS], f16, kind="ExternalInput")
    wo = nc.dram_tensor("wo", [OCOLS, D], f16, kind="ExternalInput")
    bq = nc.dram_tensor("bq", [OCOLS], f32, kind="ExternalInput")
    bk = nc.dram_tensor("bk", [OCOLS], f32, kind="ExternalInput")
    bv = nc.dram_tensor("bv", [OCOLS], f32, kind="ExternalInput")
    w_out = nc.dram_tensor("w_out", [HEADS_PER_CORE, QB, P, S], f32,
                           kind="ExternalOutput")
    o_part = nc.dram_tensor("o_part", [S, D], f32, kind="ExternalOutput")
    s_row_dram = nc.dram_tensor("s_row_scratch", [HEADS_PER_CORE, S], f32)

    Exp = mybir.ActivationFunctionType.Exp
    ADD = mybir.AluOpType.add
    MULT = mybir.AluOpType.mult

    with tile.TileContext(nc) as tc:
        with (
            tc.tile_pool(name="const", bufs=1) as const,
            tc.tile_pool(name="persist", bufs=1) as persist,
        ):
            # --- constants ---
            wq_sb = const.tile([P, KT, OCOLS], f16)
            wk_sb = const.tile([P, KT, OCOLS], f16)
            wv_sb = const.tile([P, KT, OCOLS], f16)
            wo_sb = const.tile([P, 2, D], f16)
            nc.sync.dma_start(wq_sb, wq.rearrange("(kt p) o -> p kt o", p=P))
            nc.sync.dma_start(wk_sb, wk.rearrange("(kt p) o -> p kt o", p=P))
            nc.sync.dma_start(wv_sb, wv.rearrange("(kt p) o -> p kt o", p=P))
            nc.sync.dma_start(wo_sb, wo.rearrange("(dt p) c -> p dt c", p=P))
            bq_sb = const.tile([P, 2], f32)
            bk_sb = const.tile([P, 2], f32)
            bv_bc = const.tile([P, OCOLS], f32)
            nc.sync.dma_start(bq_sb, bq.rearrange("(g p) -> p g", p=P))
            nc.sync.dma_start(bk_sb, bk.rearrange("(g p) -> p g", p=P))
            bv_ap = bv.ap()
            bv_bcast_ap = bass.AP(
                tensor=bv_ap.tensor, offset=bv_ap.offset,
                ap=[[0, P], [1, OCOLS]],
            )
            nc.sync.dma_start(bv_bc, bv_bcast_ap)
            biasm8 = const.tile([P, 1], f32)
            nc.vector.memset(biasm8, EXP_BIAS)

            # --- persistent activations ---
            # qhT/khT: head pair-tiles. tile g holds heads (2g, 2g+1) at
            # partition rows [0:64] and [64:128].
            qhT = [persist.tile([P, S], f16, name=f"qhT{g}") for g in range(2)]
            khT = [persist.tile([P, S], f16, name=f"khT{g}") for g in range(2)]
            # vh: [t-part, kblock, ocols]
            vh = persist.tile([P, KB, OCOLS], f16)
            # unnormalized attnT + scale rows
            atu = [persist.tile([P, S], f32, name=f"atu{g}") for g in range(2)]
            atn = [persist.tile([P, S], f16, name=f"atn{g}") for g in range(2)]
            s_bc = [persist.tile([P, S], f32, name=f"sbc{g}") for g in range(2)]
            s_coll = [persist.tile([P, QB], f32, name=f"scoll{h}")
                      for h in range(HEADS_PER_CORE)]

            # ---------------- Phase 1: projections ----------------
            with (
                tc.tile_pool(name="xstage", bufs=1) as xstage,
                tc.tile_pool(name="pps", bufs=1, space="PSUM") as pps,
                tc.tile_pool(name="ppsv", bufs=2, space="PSUM") as ppsv,
                tc.tile_pool(name="pco", bufs=4) as pco,
            ):
                # q and k: out qhT/khT [ocols-part, t]
                for xdram, wsb, bsb, dest in (
                    (qT, wq_sb, bq_sb, qhT),
                    (kT, wk_sb, bk_sb, khT),
                ):
                    xs = xstage.tile([P, KT, S], f16, tag="xs")
                    nc.sync.dma_start(xs, xdram.rearrange("(kt p) t -> p kt t", p=P))
                    for ob in range(2):
                        pss = [pps.tile([P, 512], f32, tag=f"pp{i}", name=f"pp{i}")
                               for i in range(4)]
                        for kt in range(KT):
                            for c in range(4):
                                nc.tensor.matmul(
                                    pss[c],
                                    wsb[:, kt, P * ob:P * (ob + 1)],
                                    xs[:, kt, 512 * c:512 * (c + 1)],
                                    start=(kt == 0), stop=(kt == KT - 1),
                                )
                        for c in range(4):
                            nc.vector.tensor_scalar(
                                out=dest[ob][:, 512 * c:512 * (c + 1)],
                                in0=pss[c], scalar1=bsb[:, ob:ob + 1],
                                scalar2=None, op0=ADD,
                            )
                # v: out vh [t-part, ocols]
                xs = xstage.tile([P, KT, S], f16, tag="xs")
                nc.sync.dma_start(xs, vT.rearrange("(kt p) t -> p kt t", p=P))
                for tb in range(KB):
                    psv = ppsv.tile([P, 512], f32, tag="ppv")
                    for kt in range(KT):
                        nc.tensor.matmul(
                            psv[:, :OCOLS],
                            xs[:, kt, P * tb:P * (tb + 1)],
                            wv_sb[:, kt, :],
                            start=(kt == 0), stop=(kt == KT - 1),
                        )
                    nc.vector.tensor_tensor(
                        out=vh[:, tb, :], in0=psv[:, :OCOLS], in1=bv_bc,
                        op=ADD,
                    )

            # ---------------- Phase 2: attention ----------------
            with (
                tc.tile_pool(name="lps", bufs=1, space="PSUM") as lps,
                tc.tile_pool(name="avps", bufs=2, space="PSUM") as avps,
                tc.tile_pool(name="wrk", bufs=3) as wrk,
                tc.tile_pool(name="wno", bufs=2) as wno,
                tc.tile_pool(name="wbtp", bufs=2) as wbtp,
                tc.tile_pool(name="small", bufs=8) as small,
            ):
                for h in range(HEADS_PER_CORE):
                    g, hh = h // 2, h % 2
                    rows = slice(64 * hh, 64 * (hh + 1))
                    qrows = qhT[g][rows, :]
                    krows = khT[g][rows, :]
                    for jj in range(QB // 4):   # groups of 4 q-blocks
                        wbt = wbtp.tile([P, KB, 512], f16, tag="wbt")
                        for j4 in range(4):
                            j = 4 * jj + j4
                            ps_log = lps.tile([P, S], f32, tag="lg")
                            for c in range(4):
                                nc.tensor.matmul(
                                    ps_log[:, 512 * c:512 * (c + 1)],
                                    qrows[:, P * j:P * (j + 1)],
                                    krows[:, 512 * c:512 * (c + 1)],
                                )
                            wexp = wrk.tile([P, S], f16, tag="wexp")
                            sums = small.tile([P, 1], f32, tag="sums")
                            nc.scalar.activation(wexp, ps_log, Exp,
                                                 bias=biasm8, accum_out=sums)
                            recip = small.tile([P, 1], f32, tag="recip")
                            nc.vector.reciprocal(recip, sums)
                            nc.vector.tensor_copy(
                                out=s_coll[h][:, j:j + 1], in_=recip)
                            wnorm = wno.tile([P, S], f32, tag="wnorm")
                            eng = nc.vector if j % 2 == 0 else nc.gpsimd
                            eng.tensor_scalar(
                                out=wnorm, in0=wexp, scalar1=recip,
                                scalar2=None, op0=MULT,
                            )
                            nc.sync.dma_start(w_out[h, j], wnorm)
                            # transposes: issued on ACT (program-ordered
                            # behind the exp that produced wexp)
                            for i in range(KB):
                                nc.scalar.dma_start_transpose(
                                    wbt[:, i, P * j4:P * (j4 + 1)],
                                    wexp[:, P * i:P * (i + 1)],
                                )
                        # AV for these 4 q-blocks (512 q columns)
                        ps_at = avps.tile([P, 512], f32, tag="av")
                        for i in range(KB):
                            nc.tensor.matmul(
                                ps_at[rows, :],
                                vh[:, i, 64 * h:64 * (h + 1)],
                                wbt[:, i, :],
                                start=(i == 0), stop=(i == KB - 1),
                                tile_position=(0, 64) if hh else None,
                            )
                        nc.scalar.copy(
                            atu[g][rows, 512 * jj:512 * (jj + 1)],
                            ps_at[rows, :],
                        )
                    # head done: bounce recip row through DRAM, broadcast
                    nc.sync.dma_start(
                        s_row_dram[h].rearrange("(j p) -> p j", p=P),
                        s_coll[h],
                    )
                    sr_ap = s_row_dram[h:h + 1, :]
                    s_bcast_ap = bass.AP(
                        tensor=sr_ap.tensor, offset=sr_ap.offset,
                        ap=[[0, 64], [1, S]],
                    )
                    nc.sync.dma_start(s_bc[g][rows, :], s_bcast_ap)
                for g in range(2):
                    nc.vector.tensor_tensor(
                        out=atn[g], in0=atu[g], in1=s_bc[g], op=MULT)

            # ---------------- Phase 3: output projection ----------------
            with (
                tc.tile_pool(name="ops", bufs=2, space="PSUM") as ops,
                tc.tile_pool(name="oco", bufs=3) as oco,
            ):
                for tb in range(KB):
                    pso = ops.tile([P, 512], f32, tag="po")
                    pso2 = ops.tile([P, 512], f32, tag="po2")
                    for dt in range(2):
                        for cc in range(2):
                            nc.tensor.matmul(
                                pso if cc == 0 else pso2,
                                atn[dt][:, P * tb:P * (tb + 1)],
                                wo_sb[:, dt, 512 * cc:512 * (cc + 1)],
                                start=(dt == 0), stop=(dt == 1),
                            )
                    osb = oco.tile([P, D], f32, tag="osb")
                    nc.scalar.copy(osb[:, :512], pso)
                    nc.vector.tensor_copy(out=osb[:, 512:], in_=pso2)
                    nc.sync.dma_start(
                        o_part[P * tb:P * (tb + 1), :], osb)

    nc.compile()
    return nc


def _get_nc():
    if "nc" not in _CACHED:
        _CACHED["nc"] = _build()
    return _CACHED["nc"]


def kernel(q, k, v, mask, Wq, bq, Wk, bk, Wv, bv, Wo, bo):
    from concourse.bass_utils import run_bass_kernel_spmd

    q = np.asarray(q); k = np.asarray(k); v = np.asarray(v)
    Wq = np.asarray(Wq); Wk = np.asarray(Wk); Wv = np.asarray(Wv)
    Wo = np.asarray(Wo)
    bq = np.asarray(bq, dtype=np.float32)
    bk = np.asarray(bk, dtype=np.float32)
    bv = np.asarray(bv, dtype=np.float32)
    bo = np.asarray(bo, dtype=np.float32)

    B = q.shape[0]
    H = 16
    nc = _get_nc()

    qT = [np.ascontiguousarray(q[b].T).astype(np.float16) for b in range(B)]
    kT = [np.ascontiguousarray(k[b].T).astype(np.float16) for b in range(B)]
    vT = [np.ascontiguousarray(v[b].T).astype(np.float16) for b in range(B)]

    in_maps = []
    for c in range(8):
        b, g = c // 4, c % 4
        sl = slice(OCOLS * g, OCOLS * (g + 1))
        in_maps.append({
            "qT": qT[b], "kT": kT[b], "vT": vT[b],
            "wq": np.ascontiguousarray(Wq[sl].T * 0.125).astype(np.float16),
            "wk": np.ascontiguousarray(Wk[sl].T).astype(np.float16),
            "wv": np.ascontiguousarray(Wv[sl].T).astype(np.float16),
            "wo": np.ascontiguousarray(Wo[:, sl].T).astype(np.float16),
            "bq": np.ascontiguousarray(bq[sl] * 0.125),
            "bk": np.ascontiguousarray(bk[sl]),
            "bv": np.ascontiguousarray(bv[sl]),
        })

    res = run_bass_kernel_spmd(nc, in_maps, core_ids=list(range(8)))
    results = res.results

    weights = np.empty((B, H, S, S), dtype=np.float32)
    out = np.zeros((B, S, D), dtype=np.float32)
    for c in range(8):
        b, g = c // 4, c % 4
        weights[b, 4 * g:4 * (g + 1)] = \
            results[c]["w_out"].reshape(HEADS_PER_CORE, S, S)
        out[b] += results[c]["o_part"]
    out += bo[None, None, :]
    return out, weights


# revision 7
# speedup vs baseline: 1.1195x; 1.1195x over previous
"""MultiHeadAttention Trainium2 kernel (8 NeuronCores).

Reference computation (B=2, S=2048, D=1024, H=16, depth=64):
    qh = split_heads(q @ Wq.T + bq); kh, vh likewise
    logits = qh @ kh.T / sqrt(64) + mask*1e-9
    weights = softmax(logits); attn = weights @ vh
    out = merge_heads(attn) @ Wo.T + bo
    returns (out, weights)

Sharding: 8 cores = 2 batches x 4 head-groups (4 heads/core).
Each core computes its batch's projections restricted to its 4 heads
(column-parallel Wq/Wk/Wv, row-parallel Wo) and a partial `out` for its
batch; host sums the 4 partials per batch and adds bo.

Device pipeline per (head, q-block of 128 rows):
  PE:   logits[128, 2048] = qhT.T @ khT   (fp16, K=64, folded 1/8 scale)
  ACT:  wexp = exp(logits - 8) -> fp16, with fp32 row-sum accumulation
  DVE:  recip = 1/sums ; wnorm = wexp * recip -> fp32 -> DMA (weights out)
  ACT-issued DMA xbar transposes: wexp[128q,128k] tiles -> wbt[k, q]
  PE:   attnT[64, q] += vh_tile.T @ wbt   (fp16, unnormalized)
  late: attn scaled by recip (broadcast via small DRAM bounce), fp16
  PE:   out_partial[t, :] = attnT.T @ WoT_slice (fp16)

Note: `mask` enters the reference as `mask * 1e-9` with mask ~ U[0,1);
its effect on the softmax is O(1e-9) relative — far below fp32 noise —
so the kernel omits it. bq/bk/bv are applied on-device; bo on host.
"""

import numpy as np

P = 128
S = 2048
D = 1024
HEADS_PER_CORE = 4
DEPTH = 64
KB = S // P       # 16 k-blocks
QB = S // P       # 16 q-blocks
KT = D // P       # 8 contraction tiles for projections
OCOLS = HEADS_PER_CORE * DEPTH  # 256 per-core projection columns
EXP_BIAS = -8.0

_CACHED = {}


def _build(repeat: int = 1):
    import concourse.bacc as bacc
    import concourse.bass as bass
    import concourse.tile as tile
    from concourse import mybir

    f16 = mybir.dt.float16
    f32 = mybir.dt.float32

    nc = bacc.Bacc("TRN2", target_bir_lowering=False)

    qT = nc.dram_tensor("qT", [D, S], f16, kind="ExternalInput")
    kT = nc.dram_tensor("kT", [D, S], f16, kind="ExternalInput")
    vT = nc.dram_tensor("vT", [D, S], f16, kind="ExternalInput")
    wq = nc.dram_tensor("wq", [D, OCOLS], f16, kind="ExternalInput")
    wk = nc.dram_tensor("wk", [D, OCOLS], f16, kind="ExternalInput")
    wv = nc.dram_tensor("wv", [D, OCOLS], f16, kind="ExternalInput")
    wo = nc.dram_tensor("wo", [OCOLS, D], f16, kind="ExternalInput")
    bq = nc.dram_tensor("bq", [OCOLS], f32, kind="ExternalInput")
    bk = nc.dram_tensor("bk", [OCOLS], f32, kind="ExternalInput")
    bv = nc.dram_tensor("bv", [OCOLS], f32, kind="ExternalInput")
    w_out = nc.dram_tensor("w_out", [HEADS_PER_CORE, QB, P, S], f32,
                           kind="ExternalOutput")
    o_part = nc.dram_tensor("o_part", [S, D], f32, kind="ExternalOutput")
    s_row_dram = nc.dram_tensor("s_row_scratch", [HEADS_PER_CORE, S], f32)

    Exp = mybir.ActivationFunctionType.Exp
    ADD = mybir.AluOpType.add
    MULT = mybir.AluOpType.mult

    import contextlib
    with tile.TileContext(nc) as tc:
        with (
            tc.tile_pool(name="const", bufs=1) as const,
            tc.tile_pool(name="persist", bufs=1) as persist,
            tc.For_i(0, repeat, 1) if repeat > 1 else contextlib.nullcontext(),
        ):
            # --- constants ---
            wq_sb = const.tile([P, KT, OCOLS], f16)
            wk_sb = const.tile([P, KT, OCOLS], f16)
            wv_sb = const.tile([P, KT, OCOLS], f16)
            wo_sb = const.tile([P, 2, D], f16)
            nc.sync.dma_start(wq_sb, wq.rearrange("(kt p) o -> p kt o", p=P))
            nc.sync.dma_start(wk_sb, wk.rearrange("(kt p) o -> p kt o", p=P))
            nc.sync.dma_start(wv_sb, wv.rearrange("(kt p) o -> p kt o", p=P))
            nc.sync.dma_start(wo_sb, wo.rearrange("(dt p) c -> p dt c", p=P))
            bq_sb = const.tile([P, 2], f32)
            bk_sb = const.tile([P, 2], f32)
            bv_bc = const.tile([P, OCOLS], f32)
            nc.sync.dma_start(bq_sb, bq.rearrange("(g p) -> p g", p=P))
            nc.sync.dma_start(bk_sb, bk.rearrange("(g p) -> p g", p=P))
            bv_ap = bv.ap()
            bv_bcast_ap = bass.AP(
                tensor=bv_ap.tensor, offset=bv_ap.offset,
                ap=[[0, P], [1, OCOLS]],
            )
            nc.sync.dma_start(bv_bc, bv_bcast_ap)
            biasm8 = const.tile([P, 1], f32)
            nc.vector.memset(biasm8, EXP_BIAS)

            # --- persistent activations ---
            # qhT/khT: head pair-tiles. tile g holds heads (2g, 2g+1) at
            # partition rows [0:64] and [64:128].
            qhT = [persist.tile([P, S], f16, name=f"qhT{g}") for g in range(2)]
            khT = [persist.tile([P, S], f16, name=f"khT{g}") for g in range(2)]
            # vh: [t-part, kblock, ocols]
            vh = persist.tile([P, KB, OCOLS], f16)
            # unnormalized attnT + scale rows
            atu = [persist.tile([P, S], f32, name=f"atu{g}") for g in range(2)]
            atn = [persist.tile([P, S], f16, name=f"atn{g}") for g in range(2)]
            s_bc = [persist.tile([P, S], f32, name=f"sbc{g}") for g in range(2)]
            s_coll = [persist.tile([P, QB], f32, name=f"scoll{h}")
                      for h in range(HEADS_PER_CORE)]

            # ---------------- Phase 1: projections ----------------
            with (
                tc.tile_pool(name="xstage", bufs=2) as xstage,
                tc.tile_pool(name="pps", bufs=1, space="PSUM") as pps,
                tc.tile_pool(name="ppsv", bufs=2, space="PSUM") as ppsv,
                tc.tile_pool(name="pco", bufs=4) as pco,
            ):
                # q and k: out qhT/khT [ocols-part, t]
                for xdram, wsb, bsb, dest in (
                    (qT, wq_sb, bq_sb, qhT),
                    (kT, wk_sb, bk_sb, khT),
                ):
                    xs = xstage.tile([P, KT, S], f16, tag="xs")
                    nc.sync.dma_start(xs, xdram.rearrange("(kt p) t -> p kt t", p=P))
                    for ob in range(2):
                        pss = [pps.tile([P, 512], f32, tag=f"pp{i}", name=f"pp{i}")
                               for i in range(4)]
                        for kt in range(KT):
                            for c in range(4):
                                nc.tensor.matmul(
                                    pss[c],
                                    wsb[:, kt, P * ob:P * (ob + 1)],
                                    xs[:, kt, 512 * c:512 * (c + 1)],
                                    start=(kt == 0), stop=(kt == KT - 1),
                                )
                        for c in range(4):
                            nc.vector.tensor_scalar(
                                out=dest[ob][:, 512 * c:512 * (c + 1)],
                                in0=pss[c], scalar1=bsb[:, ob:ob + 1],
                                scalar2=None, op0=ADD,
                            )
                # v: out vh [t-part, ocols]
                xs = xstage.tile([P, KT, S], f16, tag="xs")
                nc.sync.dma_start(xs, vT.rearrange("(kt p) t -> p kt t", p=P))
                for tb in range(KB):
                    psv = ppsv.tile([P, 512], f32, tag="ppv")
                    for kt in range(KT):
                        nc.tensor.matmul(
                            psv[:, :OCOLS],
                            xs[:, kt, P * tb:P * (tb + 1)],
                            wv_sb[:, kt, :],
                            start=(kt == 0), stop=(kt == KT - 1),
                        )
                    nc.vector.tensor_tensor(
                        out=vh[:, tb, :], in0=psv[:, :OCOLS], in1=bv_bc,
                        op=ADD,
                    )

            # ---------------- Phase 2: attention ----------------
            with (
                tc.tile_pool(name="lps", bufs=1, space="PSUM") as lps,
                tc.tile_pool(name="avps", bufs=2, space="PSUM") as avps,
                tc.tile_pool(name="wrk", bufs=3) as wrk,
                tc.tile_pool(name="wno", bufs=2) as wno,
                tc.tile_pool(name="wbtp", bufs=2) as wbtp,
                tc.tile_pool(name="small", bufs=8) as small,
            ):
                def do_av(h, jj, wbt):
                    # AV for group jj of head h (4 q-blocks, 512 q columns)
                    g, hh = h // 2, h % 2
                    rows = slice(64 * hh, 64 * (hh + 1))
                    ps_at = avps.tile([P, 512], f32, tag="av", name="av")
                    for i in range(KB):
                        nc.tensor.matmul(
                            ps_at[rows, :],
                            vh[:, i, 64 * h:64 * (h + 1)],
                            wbt[:, i, :],
                            start=(i == 0), stop=(i == KB - 1),
                            tile_position=(0, 64) if hh else None,
                        )
                    nc.scalar.copy(
                        atu[g][rows, 512 * jj:512 * (jj + 1)],
                        ps_at[rows, :],
                    )

                pending_av = None  # (h, jj, wbt) deferred by one group
                for h in range(HEADS_PER_CORE):
                    g, hh = h // 2, h % 2
                    rows = slice(64 * hh, 64 * (hh + 1))
                    qrows = qhT[g][rows, :]
                    krows = khT[g][rows, :]
                    for jj in range(QB // 4):   # groups of 4 q-blocks
                        wbt = wbtp.tile([P, KB, 512], f16, tag="wbt")
                        for j4 in range(4):
                            j = 4 * jj + j4
                            ps_log = lps.tile([P, S], f32, tag="lg")
                            for c in range(4):
                                nc.tensor.matmul(
                                    ps_log[:, 512 * c:512 * (c + 1)],
                                    qrows[:, P * j:P * (j + 1)],
                                    krows[:, 512 * c:512 * (c + 1)],
                                )
                            wexp = wrk.tile([P, S], f16, tag="wexp")
                            sums = small.tile([P, 1], f32, tag="sums")
                            nc.scalar.activation(wexp, ps_log, Exp,
                                                 bias=biasm8, accum_out=sums)
                            recip = small.tile([P, 1], f32, tag="recip")
                            nc.vector.reciprocal(recip, sums)
                            nc.vector.tensor_copy(
                                out=s_coll[h][:, j:j + 1], in_=recip)
                            wnorm = wno.tile([P, S], f32, tag="wnorm")
                            nc.vector.tensor_scalar(
                                out=wnorm, in0=wexp, scalar1=recip,
                                scalar2=None, op0=MULT,
                            )
                            nc.sync.dma_start(w_out[h, j], wnorm)
                            # one batched xbar transpose: wbt[:, i, cols j4]
                            # = wexp[:, 128i:128(i+1)].T for all 16 i
                            nc.sync.dma_start_transpose(
                                wbt[:, :, P * j4:P * (j4 + 1)], wexp)
                        if pending_av is not None:
                            do_av(*pending_av)
                        pending_av = (h, jj, wbt)
                    # head done: bounce recip row through DRAM, broadcast
                    nc.sync.dma_start(
                        s_row_dram[h].rearrange("(j p) -> p j", p=P),
                        s_coll[h],
                    )
                    sr_ap = s_row_dram[h:h + 1, :]
                    s_bcast_ap = bass.AP(
                        tensor=sr_ap.tensor, offset=sr_ap.offset,
                        ap=[[0, 64], [1, S]],
                    )
                    nc.sync.dma_start(s_bc[g][rows, :], s_bcast_ap)
                do_av(*pending_av)
                for g in range(2):
                    nc.vector.tensor_tensor(
                        out=atn[g], in0=atu[g], in1=s_bc[g], op=MULT)

            # ---------------- Phase 3: output projection ----------------
            with (
                tc.tile_pool(name="ops", bufs=2, space="PSUM") as ops,
                tc.tile_pool(name="oco", bufs=3) as oco,
            ):
                for tb in range(KB):
                    pso = ops.tile([P, 512], f32, tag="po")
                    pso2 = ops.tile([P, 512], f32, tag="po2")
                    for dt in range(2):
                        for cc in range(2):
                            nc.tensor.matmul(
                                pso if cc == 0 else pso2,
                                atn[dt][:, P * tb:P * (tb + 1)],
                                wo_sb[:, dt, 512 * cc:512 * (cc + 1)],
                                start=(dt == 0), stop=(dt == 1),
                            )
                    osb = oco.tile([P, D], f32, tag="osb")
                    nc.scalar.copy(osb[:, :512], pso)
                    nc.vector.tensor_copy(out=osb[:, 512:], in_=pso2)
                    nc.sync.dma_start(
                        o_part[P * tb:P * (tb + 1), :], osb)

    nc.compile()
    return nc


def _get_nc():
    if "nc" not in _CACHED:
        _CACHED["nc"] = _build()
    return _CACHED["nc"]


def kernel(q, k, v, mask, Wq, bq, Wk, bk, Wv, bv, Wo, bo):
    from concourse.bass_utils import run_bass_kernel_spmd

    q = np.asarray(q); k = np.asarray(k); v = np.asarray(v)
    Wq = np.asarray(Wq); Wk = np.asarray(Wk); Wv = np.asarray(Wv)
    Wo = np.asarray(Wo)
    bq = np.asarray(bq, dtype=np.float32)
    bk = np.asarray(bk, dtype=np.float32)
    bv = np.asarray(bv, dtype=np.float32)
    bo = np.asarray(bo, dtype=np.float32)

    B = q.shape[0]
    H = 16
    nc = _get_nc()

    qT = [np.ascontiguousarray(q[b].T).astype(np.float16) for b in range(B)]
    kT = [np.ascontiguousarray(k[b].T).astype(np.float16) for b in range(B)]
    vT = [np.ascontiguousarray(v[b].T).astype(np.float16) for b in range(B)]

    in_maps = []
    for c in range(8):
        b, g = c // 4, c % 4
        sl = slice(OCOLS * g, OCOLS * (g + 1))
        in_maps.append({
            "qT": qT[b], "kT": kT[b], "vT": vT[b],
            "wq": np.ascontiguousarray(Wq[sl].T * 0.125).astype(np.float16),
            "wk": np.ascontiguousarray(Wk[sl].T).astype(np.float16),
            "wv": np.ascontiguousarray(Wv[sl].T).astype(np.float16),
            "wo": np.ascontiguousarray(Wo[:, sl].T).astype(np.float16),
            "bq": np.ascontiguousarray(bq[sl] * 0.125),
            "bk": np.ascontiguousarray(bk[sl]),
            "bv": np.ascontiguousarray(bv[sl]),
        })

    res = run_bass_kernel_spmd(nc, in_maps, core_ids=list(range(8)))
    results = res.results

    weights = np.empty((B, H, S, S), dtype=np.float32)
    out = np.zeros((B, S, D), dtype=np.float32)
    for c in range(8):
        b, g = c // 4, c % 4
        weights[b, 4 * g:4 * (g + 1)] = \
            results[c]["w_out"].reshape(HEADS_PER_CORE, S, S)
        out[b] += results[c]["o_part"]
    out += bo[None, None, :]
    return out, weights
